# revision 12
# baseline (speedup 1.0000x reference)
"""Causal multi-head attention (B=4, S=2048, HID=1024, 16 heads x 64) with RoPE
on 8 TRN2 NeuronCores.

Sharding: core c -> batch b = c//2, head-group hg = c%2 (8 heads each).

Per core: projections in transposed layout (head dims on partitions), RoPE
rotation realized as a strided-partition DMA permutation (sign folded into the
sin table), scores computed transposed sT[kk, q] with both heads of a pair
row-packed into one PSUM tile, exp on ACT with fused scale, causal masking via
affine_select (exact zeros), V augmented with a ones column so the softmax
denominator appears as ctx psum row 64, deferred normalization (sums -> DRAM
bounce -> broadcast DMA -> fast reciprocal -> multiply), o_proj and a pair
ReduceScatter pipelined per 512-query chunk.

All matmuls run in fp16 (fp32 PSUM accumulation).
"""
import os as _os
import numpy as np
from contextlib import ExitStack

import concourse.bass as bass
import concourse.tile as tile
import concourse.mybir as mybir
from concourse import bacc
from concourse.alu_op_type import AluOpType
from concourse.bass_utils import run_bass_kernel_spmd

F32 = mybir.dt.float32
F16 = mybir.dt.float16
BF16 = mybir.dt.bfloat16
MM_DT = BF16 if _os.environ.get("KMM", "f16") == "bf16" else F16
ROT_DMA = _os.environ.get("KROT", "dma") == "dma"
AF = mybir.ActivationFunctionType
Alu = AluOpType

B, S, HID = 4, 2048, 1024
NH, HD = 16, 64
SCALE = 1.0 / np.sqrt(HD)
ROPE_BASE = 10000.0
NCORES = 8
HPC = 8          # heads per core
JC = 512         # head dims per core
NJ = 4           # q chunks of 512
NT = 16          # kk tiles of 128
NSC = 4          # s chunks of 512 for projections
NHC = 8          # hid chunks of 128 (contraction)

_PROGRAM = None


def build():
    nc = bacc.Bacc("TRN2", target_bir_lowering=False, debug=False)

    hsT_d = nc.declare_dram_parameter("hsT", [HID, S], MM_DT, isOutput=False)
    wq_d = nc.declare_dram_parameter("wqT", [HID, JC], MM_DT, isOutput=False)
    wk_d = nc.declare_dram_parameter("wkT", [HID, JC], MM_DT, isOutput=False)
    wv_d = nc.declare_dram_parameter("wvT", [HID, JC], MM_DT, isOutput=False)
    if not ROT_DMA:
        wqr_d = nc.declare_dram_parameter("wqrT", [HID, JC], MM_DT, isOutput=False)
        wkr_d = nc.declare_dram_parameter("wkrT", [HID, JC], MM_DT, isOutput=False)
    wo_d = nc.declare_dram_parameter("woT", [JC, HID], MM_DT, isOutput=False)
    cos_d = nc.declare_dram_parameter("cosT2", [128, S], MM_DT, isOutput=False)
    sin_d = nc.declare_dram_parameter("sinT2", [128, S], MM_DT, isOutput=False)
    out_d = nc.declare_dram_parameter("out", [S // 2, HID], F32, isOutput=True)

    rdram = nc.dram_tensor("rdram", [HPC, S], F32)
    cc_in = nc.dram_tensor("cc_in", [S, HID], F32)
    cc_out = [nc.dram_tensor(f"cc_out{j}", [S // 8, HID], F32) for j in range(NJ)]

    with ExitStack() as ctx:
        tc = ctx.enter_context(tile.TileContext(nc, num_cores=NCORES))
        consts = ctx.enter_context(tc.tile_pool(name="consts", bufs=1))
        rt = ctx.enter_context(tc.tile_pool(name="rt", bufs=6))
        ptp = ctx.enter_context(tc.tile_pool(name="ptp", bufs=6))
        misc = ctx.enter_context(tc.tile_pool(name="misc", bufs=2))
        outp = ctx.enter_context(tc.tile_pool(name="outp", bufs=3))
        psum = ctx.enter_context(tc.tile_pool(name="psum", bufs=2, space="PSUM"))

        # ---- load constants ----
        hsT = consts.tile([128, NHC, S], MM_DT, tag="hsT")
        for hc in range(NHC):
            nc.sync.dma_start(out=hsT[:, hc, :], in_=hsT_d[hc * 128:(hc + 1) * 128, :])
        wsb = {}
        wnames = [("wq", wq_d), ("wk", wk_d), ("wv", wv_d)]
        if not ROT_DMA:
            wnames += [("wqr", wqr_d), ("wkr", wkr_d)]
        for name, d in wnames:
            t = consts.tile([128, NHC, JC], MM_DT, tag=name, name=f"w_{name}")
            for hc in range(NHC):
                nc.sync.dma_start(out=t[:, hc, :], in_=d[hc * 128:(hc + 1) * 128, :])
            wsb[name] = t
        wo = consts.tile([128, 4, HID], MM_DT, tag="wo")
        nc.sync.dma_start(out=wo[:], in_=wo_d[:].rearrange("(c p) j -> p c j", p=128))
        cos2 = consts.tile([128, S], MM_DT, tag="cos2")
        sin2 = consts.tile([128, S], MM_DT, tag="sin2")
        nc.sync.dma_start(out=cos2[:], in_=cos_d[:])
        nc.sync.dma_start(out=sin2[:], in_=sin_d[:])

        # ---- projections + RoPE for ALL head pairs up front ----
        qrope = [consts.tile([128, S], MM_DT, tag=f"qrope{i}", name=f"qrope{i}")
                 for i in range(4)]
        krope = [consts.tile([128, S], MM_DT, tag=f"krope{i}", name=f"krope{i}")
                 for i in range(4)]

        for hp in range(4):
            jcol = hp * 128
            for wname, raww, dest in (("wq", "qraw", qrope), ("wk", "kraw", krope)):
                for sc in range(NSC):
                    ssl = slice(sc * 512, (sc + 1) * 512)
                    ps_raw = psum.tile([128, 512], F32, tag="mm")
                    for hc in range(NHC):
                        nc.tensor.matmul(
                            out=ps_raw[:],
                            lhsT=wsb[wname][:, hc, jcol:jcol + 128],
                            rhs=hsT[:, hc, ssl],
                            start=(hc == 0), stop=(hc == NHC - 1),
                        )
                    if ROT_DMA:
                        raw_sb = misc.tile([128, 512], MM_DT, tag="qraw", bufs=4,
                                           name=f"raw_{raww}{hp}_{sc}")
                        nc.scalar.copy(out=raw_sb[:], in_=ps_raw[:])
                        rot_sb = misc.tile([128, 512], MM_DT, tag="qrot", bufs=4,
                                           name=f"rot_{raww}{hp}_{sc}")
                        for hl in range(2):
                            b0 = 64 * hl
                            # rot rows 0:32 <- raw rows 1,3,..,63 (odd)
                            nc.sync.dma_start(
                                out=rot_sb[b0:b0 + 32, :],
                                in_=raw_sb[b0 + 1:b0 + 64:2, :],
                            )
                            # rot rows 32:64 <- raw rows 0,2,..,62 (even)
                            nc.sync.dma_start(
                                out=rot_sb[b0 + 32:b0 + 64, :],
                                in_=raw_sb[b0:b0 + 63:2, :],
                            )
                        t1 = rt.tile([128, 512], MM_DT, tag="rt")
                        t2 = rt.tile([128, 512], MM_DT, tag="rt")
                        nc.vector.tensor_tensor(out=t1[:], in0=raw_sb[:], in1=cos2[:, ssl], op=Alu.mult)
                        nc.vector.tensor_tensor(out=t2[:], in0=rot_sb[:], in1=sin2[:, ssl], op=Alu.mult)
                        nc.vector.tensor_add(out=dest[hp][:, ssl], in0=t1[:], in1=t2[:])
                    else:
                        rot_w = {"wq": "wqr", "wk": "wkr"}[wname]
                        ps_rot = psum.tile([128, 512], F32, tag="mm")
                        for hc in range(NHC):
                            nc.tensor.matmul(
                                out=ps_rot[:],
                                lhsT=wsb[rot_w][:, hc, jcol:jcol + 128],
                                rhs=hsT[:, hc, ssl],
                                start=(hc == 0), stop=(hc == NHC - 1),
                            )
                        t1 = rt.tile([128, 512], F32, tag="rt")
                        t2 = rt.tile([128, 512], F32, tag="rt")
                        nc.vector.tensor_tensor(out=t1[:], in0=ps_raw[:], in1=cos2[:, ssl], op=Alu.mult)
                        nc.vector.tensor_tensor(out=t2[:], in0=ps_rot[:], in1=sin2[:, ssl], op=Alu.mult)
                        nc.vector.tensor_add(out=dest[hp][:, ssl], in0=t1[:], in1=t2[:])

        # ---- V for all heads, natural layout + ones column ----
        v_sb = consts.tile([128, NT, HPC, HD + 1], MM_DT, tag="v_sb")
        nc.vector.memset(v_sb[:, :, :, HD:HD + 1], 1.0)
        for st in range(NT):
            v_ps = psum.tile([128, JC], F32, tag="mm")
            for hc in range(NHC):
                nc.tensor.matmul(
                    out=v_ps[:],
                    lhsT=hsT[:, hc, st * 128:(st + 1) * 128],
                    rhs=wsb["wv"][:, hc, :],
                    start=(hc == 0), stop=(hc == NHC - 1),
                )
            nc.vector.tensor_copy(
                out=v_sb[:, st, :, 0:HD],
                in_=v_ps[:].rearrange("p (h d) -> p h d", h=HPC),
            )

        # ---- attention + o_proj + collective, pipelined per q-chunk ----
        ctx_sb = [consts.tile([128, S], MM_DT, tag=f"ctx{i}", name=f"ctx_sb{i}")
                  for i in range(4)]

        for j in range(NJ):
            qsl = slice(j * 512, (j + 1) * 512)
            for hp in range(4):
                ctx_ps = [psum.tile([HD + 1, 512], F32, tag="ctx", name=f"ctx_ps{_i}")
                          for _i in range(2)]
                nt = 4 * j + 4
                for t in range(nt):
                    sc_ps = psum.tile([128, 2, 512], F32, tag="sc")
                    ksl = slice(t * 128, (t + 1) * 128)
                    for hl in range(2):
                        pr = slice(64 * hl, 64 * hl + 64)
                        nc.tensor.matmul(
                            out=sc_ps[:, hl, :],
                            lhsT=krope[hp][pr, ksl],
                            rhs=qrope[hp][pr, qsl],
                            start=True, stop=True,
                        )
                    pt = ptp.tile([128, 2, 512], MM_DT, tag="pt")
                    nc.scalar.activation(out=pt[:], in_=sc_ps[:], func=AF.Exp, scale=float(SCALE))
                    if t >= 4 * j:
                        for hl in range(2):
                            nc.gpsimd.affine_select(
                                out=pt[:, hl, :], in_=pt[:, hl, :],
                                pattern=[[1, 512]], compare_op=Alu.is_ge,
                                fill=0.0, base=512 * j - 128 * t,
                                channel_multiplier=-1,
                            )
                    for hl in range(2):
                        nc.tensor.matmul(
                            out=ctx_ps[hl][:],
                            lhsT=v_sb[:, t, 2 * hp + hl, :],
                            rhs=pt[:, hl, :],
                            start=(t == 0), stop=(t == nt - 1),
                        )
                for hl in range(2):
                    pr = slice(64 * hl, 64 * hl + 64)
                    nc.vector.tensor_copy(out=ctx_sb[hp][pr, qsl], in_=ctx_ps[hl][0:HD, :])
                    srow = misc.tile([128, 512], F32, tag="srow")
                    nc.vector.tensor_copy(out=srow[64:65, :], in_=ctx_ps[hl][HD:HD + 1, :])
                    nc.sync.dma_start(
                        out=rdram[2 * hp + hl:2 * hp + hl + 1, qsl],
                        in_=srow[64:65, :],
                    )
            # normalize all head pairs for this q chunk
            for hp in range(4):
                bc = misc.tile([128, 512], F32, tag="bc")
                nc.sync.dma_start(
                    out=bc[0:64, :],
                    in_=rdram[2 * hp:2 * hp + 1, qsl].partition_broadcast(64),
                )
                nc.sync.dma_start(
                    out=bc[64:128, :],
                    in_=rdram[2 * hp + 1:2 * hp + 2, qsl].partition_broadcast(64),
                )
                nc.vector.reciprocal_approx_fast(out=bc[:], in_=bc[:])
                nc.vector.tensor_tensor(
                    out=ctx_sb[hp][:, qsl], in0=ctx_sb[hp][:, qsl], in1=bc[:], op=Alu.mult,
                )
            # o_proj rows of this q chunk
            for st in range(4 * j, 4 * j + 4):
                ssl2 = slice(st * 128, (st + 1) * 128)
                for jc2 in range(2):
                    osl = slice(jc2 * 512, (jc2 + 1) * 512)
                    o_ps = psum.tile([128, 512], F32, tag="mm")
                    for kc in range(4):
                        nc.tensor.matmul(
                            out=o_ps[:],
                            lhsT=ctx_sb[kc][:, ssl2],
                            rhs=wo[:, kc, osl],
                            start=(kc == 0), stop=(kc == 3),
                        )
                    o_sb = outp.tile([128, 512], F32, tag="osb")
                    nc.vector.tensor_copy(out=o_sb[:], in_=o_ps[:])
                    nc.sync.dma_start(out=cc_in[ssl2, osl], in_=o_sb[:])
            # pair ReduceScatter for this q chunk + final store
            nc.gpsimd.collective_compute(
                "ReduceScatter", Alu.add,
                replica_groups=[[0, 1], [2, 3], [4, 5], [6, 7]],
                ins=[cc_in[j * 512:(j + 1) * 512, :]], outs=[cc_out[j][:]],
            )

        for j in range(NJ):
            nc.sync.dma_start(
                out=out_d[j * 256:(j + 1) * 256, :], in_=cc_out[j][:],
            )

    nc.finalize()
    return nc


def _rope_tables():
    inv_freq = (1.0 / (ROPE_BASE ** (np.arange(0, HD, 2, dtype=np.float32) / np.float32(HD)))).astype(np.float32)
    t = np.arange(S, dtype=np.float32)
    freqs = np.outer(t, inv_freq).astype(np.float32)          # [S, 32]
    emb = np.concatenate([freqs, freqs], axis=-1)             # [S, 64]
    return np.cos(emb).astype(np.float32), np.sin(emb).astype(np.float32)


def _rot_weights(W):
    """Rows of Wr give rotated(x) = cat(-x2, x1) of x = W @ h per 64-dim head."""
    Wr = np.empty_like(W)
    for h in range(NH):
        b = h * HD
        Wr[b:b + 32] = -W[b + 1:b + HD:2]
        Wr[b + 32:b + HD] = W[b:b + HD:2]
    return Wr


def prepare_in_maps(hidden_states, Wq, Wk, Wv, Wo):
    cos, sin = _rope_tables()                                  # [S, 64]
    cos2 = np.ascontiguousarray(np.tile(cos.T, (2, 1)))        # [128, S]
    sin2 = np.ascontiguousarray(np.tile(sin.T, (2, 1)))
    if ROT_DMA:
        # sign of the rotation (-x2 for d<32) folded into the sin table
        sin2[0:32] *= -1.0
        sin2[64:96] *= -1.0
    else:
        Wqr = _rot_weights(Wq)
        Wkr = _rot_weights(Wk)
    if MM_DT == F16:
        f16 = np.float16
    else:
        import ml_dtypes
        f16 = ml_dtypes.bfloat16
    in_maps = []
    for c in range(NCORES):
        b, hg = c // 2, c % 2
        sl = slice(JC * hg, JC * (hg + 1))
        m = {
            "hsT": np.ascontiguousarray(hidden_states[b].T).astype(f16),
            "wqT": np.ascontiguousarray(Wq[sl].T).astype(f16),
            "wkT": np.ascontiguousarray(Wk[sl].T).astype(f16),
            "wvT": np.ascontiguousarray(Wv[sl].T).astype(f16),
            "woT": np.ascontiguousarray(Wo[:, sl].T).astype(f16),
            "cosT2": cos2.astype(f16),
            "sinT2": sin2.astype(f16),
        }
        if not ROT_DMA:
            m["wqrT"] = np.ascontiguousarray(Wqr[sl].T).astype(f16)
            m["wkrT"] = np.ascontiguousarray(Wkr[sl].T).astype(f16)
        in_maps.append(m)
    return in_maps


def run(inputs, trace=False, tmpdir=None):
    global _PROGRAM
    if _PROGRAM is None:
        _PROGRAM = build()
    nc = _PROGRAM
    in_maps = prepare_in_maps(
        np.asarray(inputs["hidden_states"], dtype=np.float32),
        np.asarray(inputs["Wq"], dtype=np.float32),
        np.asarray(inputs["Wk"], dtype=np.float32),
        np.asarray(inputs["Wv"], dtype=np.float32),
        np.asarray(inputs["Wo"], dtype=np.float32),
    )
    res = run_bass_kernel_spmd(nc, in_maps, list(range(NCORES)), trace=trace, tmpdir=tmpdir)
    out = np.empty((B, S, HID), dtype=np.float32)
    for b in range(B):
        lo, hi = res.results[2 * b]["out"], res.results[2 * b + 1]["out"]
        for j in range(NJ):
            out[b, 512 * j:512 * j + 256] = lo[256 * j:256 * (j + 1)]
            out[b, 512 * j + 256:512 * (j + 1)] = hi[256 * j:256 * (j + 1)]
    return out, res


def kernel(**inputs):
    out, _ = run(inputs)
    return out


# revision 13
# speedup vs baseline: 1.0271x; 1.0271x over previous
"""Causal multi-head attention (B=4, S=2048, HID=1024, 16 heads x 64) with RoPE
on 8 TRN2 NeuronCores.

Sharding: core c -> batch b = c//2, head-group hg = c%2 (8 heads each).

Per core: projections in transposed layout (head dims on partitions), RoPE
rotation realized as a strided-partition DMA permutation (sign folded into the
sin table), scores computed transposed sT[kk, q] with both heads of a pair
row-packed into one PSUM tile, exp on ACT with fused scale, causal masking via
affine_select (exact zeros), V augmented with a ones column so the softmax
denominator appears as ctx psum row 64, deferred normalization (sums -> DRAM
bounce -> broadcast DMA -> fast reciprocal -> multiply), o_proj and a pair
ReduceScatter pipelined per 512-query chunk.

All matmuls run in fp16 (fp32 PSUM accumulation).
"""
import os as _os
import numpy as np
from contextlib import ExitStack

import concourse.bass as bass
import concourse.tile as tile
import concourse.mybir as mybir
from concourse import bacc
from concourse.alu_op_type import AluOpType
from concourse.bass_utils import run_bass_kernel_spmd
import concourse.bass_utils as _bu

LDW_OPT = _os.environ.get("KLDW", "0") == "1"
if LDW_OPT and not getattr(_bu, "_ldw_patched", False):
    _orig_run_command = _bu.run_command
    def _run_command_ldwopt(argv, **kw):
        argv = ["--enable-ldw-opt=true" if a == "--enable-ldw-opt=false" else a
                for a in argv]
        return _orig_run_command(argv, **kw)
    _bu.run_command = _run_command_ldwopt
    _bu._ldw_patched = True

F32 = mybir.dt.float32
F16 = mybir.dt.float16
BF16 = mybir.dt.bfloat16
MM_DT = BF16 if _os.environ.get("KMM", "f16") == "bf16" else F16
ROT_DMA = _os.environ.get("KROT", "dma") == "dma"
AF = mybir.ActivationFunctionType
Alu = AluOpType

B, S, HID = 4, 2048, 1024
NH, HD = 16, 64
SCALE = 1.0 / np.sqrt(HD)
ROPE_BASE = 10000.0
NCORES = 8
HPC = 8          # heads per core
JC = 512         # head dims per core
NJ = 4           # q chunks of 512
NT = 16          # kk tiles of 128
NSC = 4          # s chunks of 512 for projections
NHC = 8          # hid chunks of 128 (contraction)

_PROGRAM = None


def build():
    nc = bacc.Bacc("TRN2", target_bir_lowering=False, debug=False)

    hsT_d = nc.declare_dram_parameter("hsT", [HID, S], MM_DT, isOutput=False)
    wq_d = nc.declare_dram_parameter("wqT", [HID, JC], MM_DT, isOutput=False)
    wk_d = nc.declare_dram_parameter("wkT", [HID, JC], MM_DT, isOutput=False)
    wv_d = nc.declare_dram_parameter("wvT", [HID, JC], MM_DT, isOutput=False)
    if not ROT_DMA:
        wqr_d = nc.declare_dram_parameter("wqrT", [HID, JC], MM_DT, isOutput=False)
        wkr_d = nc.declare_dram_parameter("wkrT", [HID, JC], MM_DT, isOutput=False)
    wo_d = nc.declare_dram_parameter("woT", [JC, HID], MM_DT, isOutput=False)
    cos_d = nc.declare_dram_parameter("cosT2", [128, S], MM_DT, isOutput=False)
    sin_d = nc.declare_dram_parameter("sinT2", [128, S], MM_DT, isOutput=False)
    out_d = nc.declare_dram_parameter("out", [S // 2, HID], F32, isOutput=True)

    rdram = nc.dram_tensor("rdram_l" if LDW_OPT else "rdram", [HPC, S], F32)
    cc_in = nc.dram_tensor("cc_in", [S, HID], F32)
    cc_out = [nc.dram_tensor(f"cc_out{j}", [S // 8, HID], F32) for j in range(NJ)]

    with ExitStack() as ctx:
        tc = ctx.enter_context(tile.TileContext(nc, num_cores=NCORES))
        consts = ctx.enter_context(tc.tile_pool(name="consts", bufs=1))
        rt = ctx.enter_context(tc.tile_pool(name="rt", bufs=6))
        ptp = ctx.enter_context(tc.tile_pool(name="ptp", bufs=6))
        misc = ctx.enter_context(tc.tile_pool(name="misc", bufs=2))
        outp = ctx.enter_context(tc.tile_pool(name="outp", bufs=3))
        psum = ctx.enter_context(tc.tile_pool(name="psum", bufs=2, space="PSUM"))

        # ---- load constants ----
        hsT = consts.tile([128, NHC, S], MM_DT, tag="hsT")
        for hc in range(NHC):
            nc.sync.dma_start(out=hsT[:, hc, :], in_=hsT_d[hc * 128:(hc + 1) * 128, :])
        wsb = {}
        wnames = [("wq", wq_d), ("wk", wk_d), ("wv", wv_d)]
        if not ROT_DMA:
            wnames += [("wqr", wqr_d), ("wkr", wkr_d)]
        for name, d in wnames:
            t = consts.tile([128, NHC, JC], MM_DT, tag=name, name=f"w_{name}")
            for hc in range(NHC):
                nc.sync.dma_start(out=t[:, hc, :], in_=d[hc * 128:(hc + 1) * 128, :])
            wsb[name] = t
        wo = consts.tile([128, 4, HID], MM_DT, tag="wo")
        nc.sync.dma_start(out=wo[:], in_=wo_d[:].rearrange("(c p) j -> p c j", p=128))
        cos2 = consts.tile([128, S], MM_DT, tag="cos2")
        sin2 = consts.tile([128, S], MM_DT, tag="sin2")
        nc.sync.dma_start(out=cos2[:], in_=cos_d[:])
        nc.sync.dma_start(out=sin2[:], in_=sin_d[:])

        # ---- projections + RoPE for ALL head pairs up front ----
        qrope = [consts.tile([128, S], MM_DT, tag=f"qrope{i}", name=f"qrope{i}")
                 for i in range(4)]
        krope = [consts.tile([128, S], MM_DT, tag=f"krope{i}", name=f"krope{i}")
                 for i in range(4)]

        for hp in range(4):
            jcol = hp * 128
            for wname, raww, dest in (("wq", "qraw", qrope), ("wk", "kraw", krope)):
                for sc in range(NSC):
                    ssl = slice(sc * 512, (sc + 1) * 512)
                    ps_raw = psum.tile([128, 512], F32, tag="mm")
                    for hc in range(NHC):
                        nc.tensor.matmul(
                            out=ps_raw[:],
                            lhsT=wsb[wname][:, hc, jcol:jcol + 128],
                            rhs=hsT[:, hc, ssl],
                            start=(hc == 0), stop=(hc == NHC - 1),
                        )
                    if ROT_DMA:
                        raw_sb = misc.tile([128, 512], MM_DT, tag="qraw", bufs=4,
                                           name=f"raw_{raww}{hp}_{sc}")
                        nc.scalar.copy(out=raw_sb[:], in_=ps_raw[:])
                        rot_sb = misc.tile([128, 512], MM_DT, tag="qrot", bufs=4,
                                           name=f"rot_{raww}{hp}_{sc}")
                        for hl in range(2):
                            b0 = 64 * hl
                            # rot rows 0:32 <- raw rows 1,3,..,63 (odd)
                            nc.sync.dma_start(
                                out=rot_sb[b0:b0 + 32, :],
                                in_=raw_sb[b0 + 1:b0 + 64:2, :],
                            )
                            # rot rows 32:64 <- raw rows 0,2,..,62 (even)
                            nc.sync.dma_start(
                                out=rot_sb[b0 + 32:b0 + 64, :],
                                in_=raw_sb[b0:b0 + 63:2, :],
                            )
                        t1 = rt.tile([128, 512], MM_DT, tag="rt")
                        t2 = rt.tile([128, 512], MM_DT, tag="rt")
                        nc.vector.tensor_tensor(out=t1[:], in0=raw_sb[:], in1=cos2[:, ssl], op=Alu.mult)
                        nc.vector.tensor_tensor(out=t2[:], in0=rot_sb[:], in1=sin2[:, ssl], op=Alu.mult)
                        nc.vector.tensor_add(out=dest[hp][:, ssl], in0=t1[:], in1=t2[:])
                    else:
                        rot_w = {"wq": "wqr", "wk": "wkr"}[wname]
                        ps_rot = psum.tile([128, 512], F32, tag="mm")
                        for hc in range(NHC):
                            nc.tensor.matmul(
                                out=ps_rot[:],
                                lhsT=wsb[rot_w][:, hc, jcol:jcol + 128],
                                rhs=hsT[:, hc, ssl],
                                start=(hc == 0), stop=(hc == NHC - 1),
                            )
                        t1 = rt.tile([128, 512], F32, tag="rt")
                        t2 = rt.tile([128, 512], F32, tag="rt")
                        nc.vector.tensor_tensor(out=t1[:], in0=ps_raw[:], in1=cos2[:, ssl], op=Alu.mult)
                        nc.vector.tensor_tensor(out=t2[:], in0=ps_rot[:], in1=sin2[:, ssl], op=Alu.mult)
                        nc.vector.tensor_add(out=dest[hp][:, ssl], in0=t1[:], in1=t2[:])

        # ---- V for all heads, natural layout + ones column ----
        v_sb = consts.tile([128, NT, HPC, HD + 1], MM_DT, tag="v_sb")
        nc.vector.memset(v_sb[:, :, :, HD:HD + 1], 1.0)
        for st in range(NT):
            v_ps = psum.tile([128, JC], F32, tag="mm")
            for hc in range(NHC):
                nc.tensor.matmul(
                    out=v_ps[:],
                    lhsT=hsT[:, hc, st * 128:(st + 1) * 128],
                    rhs=wsb["wv"][:, hc, :],
                    start=(hc == 0), stop=(hc == NHC - 1),
                )
            nc.vector.tensor_copy(
                out=v_sb[:, st, :, 0:HD],
                in_=v_ps[:].rearrange("p (h d) -> p h d", h=HPC),
            )

        # ---- attention + o_proj + collective, pipelined per q-chunk ----
        ctx_sb = [consts.tile([128, S], MM_DT, tag=f"ctx{i}", name=f"ctx_sb{i}")
                  for i in range(4)]

        for j in range(NJ):
            qsl = slice(j * 512, (j + 1) * 512)
            for hp in range(4):
                ctx_ps = [psum.tile([HD + 1, 512], F32, tag="ctx", name=f"ctx_ps{_i}")
                          for _i in range(2)]
                nt = 4 * j + 4
                for t in range(nt):
                    sc_ps = psum.tile([128, 2, 512], F32, tag="sc")
                    ksl = slice(t * 128, (t + 1) * 128)
                    for hl in range(2):
                        pr = slice(64 * hl, 64 * hl + 64)
                        nc.tensor.matmul(
                            out=sc_ps[:, hl, :],
                            lhsT=krope[hp][pr, ksl],
                            rhs=qrope[hp][pr, qsl],
                            start=True, stop=True,
                        )
                    pt = ptp.tile([128, 2, 512], MM_DT, tag="pt")
                    nc.scalar.activation(out=pt[:], in_=sc_ps[:], func=AF.Exp, scale=float(SCALE))
                    if t >= 4 * j:
                        for hl in range(2):
                            nc.gpsimd.affine_select(
                                out=pt[:, hl, :], in_=pt[:, hl, :],
                                pattern=[[1, 512]], compare_op=Alu.is_ge,
                                fill=0.0, base=512 * j - 128 * t,
                                channel_multiplier=-1,
                            )
                    for hl in range(2):
                        nc.tensor.matmul(
                            out=ctx_ps[hl][:],
                            lhsT=v_sb[:, t, 2 * hp + hl, :],
                            rhs=pt[:, hl, :],
                            start=(t == 0), stop=(t == nt - 1),
                        )
                for hl in range(2):
                    pr = slice(64 * hl, 64 * hl + 64)
                    nc.vector.tensor_copy(out=ctx_sb[hp][pr, qsl], in_=ctx_ps[hl][0:HD, :])
                    srow = misc.tile([128, 512], F32, tag="srow")
                    nc.vector.tensor_copy(out=srow[64:65, :], in_=ctx_ps[hl][HD:HD + 1, :])
                    nc.sync.dma_start(
                        out=rdram[2 * hp + hl:2 * hp + hl + 1, qsl],
                        in_=srow[64:65, :],
                    )
            # normalize all head pairs for this q chunk
            for hp in range(4):
                bc = misc.tile([128, 512], F32, tag="bc")
                nc.sync.dma_start(
                    out=bc[0:64, :],
                    in_=rdram[2 * hp:2 * hp + 1, qsl].partition_broadcast(64),
                )
                nc.sync.dma_start(
                    out=bc[64:128, :],
                    in_=rdram[2 * hp + 1:2 * hp + 2, qsl].partition_broadcast(64),
                )
                nc.vector.reciprocal_approx_fast(out=bc[:], in_=bc[:])
                nc.vector.tensor_tensor(
                    out=ctx_sb[hp][:, qsl], in0=ctx_sb[hp][:, qsl], in1=bc[:], op=Alu.mult,
                )
            # o_proj rows of this q chunk
            for st in range(4 * j, 4 * j + 4):
                ssl2 = slice(st * 128, (st + 1) * 128)
                for jc2 in range(2):
                    osl = slice(jc2 * 512, (jc2 + 1) * 512)
                    o_ps = psum.tile([128, 512], F32, tag="mm")
                    for kc in range(4):
                        nc.tensor.matmul(
                            out=o_ps[:],
                            lhsT=ctx_sb[kc][:, ssl2],
                            rhs=wo[:, kc, osl],
                            start=(kc == 0), stop=(kc == 3),
                        )
                    o_sb = outp.tile([128, 512], F32, tag="osb")
                    nc.vector.tensor_copy(out=o_sb[:], in_=o_ps[:])
                    nc.sync.dma_start(out=cc_in[ssl2, osl], in_=o_sb[:])
            # pair ReduceScatter for this q chunk + final store
            nc.gpsimd.collective_compute(
                "ReduceScatter", Alu.add,
                replica_groups=[[0, 1], [2, 3], [4, 5], [6, 7]],
                ins=[cc_in[j * 512:(j + 1) * 512, :]], outs=[cc_out[j][:]],
            )

        for j in range(NJ):
            nc.sync.dma_start(
                out=out_d[j * 256:(j + 1) * 256, :], in_=cc_out[j][:],
            )

    nc.finalize()
    return nc


def _rope_tables():
    inv_freq = (1.0 / (ROPE_BASE ** (np.arange(0, HD, 2, dtype=np.float32) / np.float32(HD)))).astype(np.float32)
    t = np.arange(S, dtype=np.float32)
    freqs = np.outer(t, inv_freq).astype(np.float32)          # [S, 32]
    emb = np.concatenate([freqs, freqs], axis=-1)             # [S, 64]
    return np.cos(emb).astype(np.float32), np.sin(emb).astype(np.float32)


def _rot_weights(W):
    """Rows of Wr give rotated(x) = cat(-x2, x1) of x = W @ h per 64-dim head."""
    Wr = np.empty_like(W)
    for h in range(NH):
        b = h * HD
        Wr[b:b + 32] = -W[b + 1:b + HD:2]
        Wr[b + 32:b + HD] = W[b:b + HD:2]
    return Wr


def prepare_in_maps(hidden_states, Wq, Wk, Wv, Wo):
    cos, sin = _rope_tables()                                  # [S, 64]
    cos2 = np.ascontiguousarray(np.tile(cos.T, (2, 1)))        # [128, S]
    sin2 = np.ascontiguousarray(np.tile(sin.T, (2, 1)))
    if ROT_DMA:
        # sign of the rotation (-x2 for d<32) folded into the sin table
        sin2[0:32] *= -1.0
        sin2[64:96] *= -1.0
    else:
        Wqr = _rot_weights(Wq)
        Wkr = _rot_weights(Wk)
    if MM_DT == F16:
        f16 = np.float16
    else:
        import ml_dtypes
        f16 = ml_dtypes.bfloat16
    in_maps = []
    for c in range(NCORES):
        b, hg = c // 2, c % 2
        sl = slice(JC * hg, JC * (hg + 1))
        m = {
            "hsT": np.ascontiguousarray(hidden_states[b].T).astype(f16),
            "wqT": np.ascontiguousarray(Wq[sl].T).astype(f16),
            "wkT": np.ascontiguousarray(Wk[sl].T).astype(f16),
            "wvT": np.ascontiguousarray(Wv[sl].T).astype(f16),
            "woT": np.ascontiguousarray(Wo[:, sl].T).astype(f16),
            "cosT2": cos2.astype(f16),
            "sinT2": sin2.astype(f16),
        }
        if not ROT_DMA:
            m["wqrT"] = np.ascontiguousarray(Wqr[sl].T).astype(f16)
            m["wkrT"] = np.ascontiguousarray(Wkr[sl].T).astype(f16)
        in_maps.append(m)
    return in_maps


def run(inputs, trace=False, tmpdir=None):
    global _PROGRAM
    if _PROGRAM is None:
        _PROGRAM = build()
    nc = _PROGRAM
    in_maps = prepare_in_maps(
        np.asarray(inputs["hidden_states"], dtype=np.float32),
        np.asarray(inputs["Wq"], dtype=np.float32),
        np.asarray(inputs["Wk"], dtype=np.float32),
        np.asarray(inputs["Wv"], dtype=np.float32),
        np.asarray(inputs["Wo"], dtype=np.float32),
    )
    res = run_bass_kernel_spmd(nc, in_maps, list(range(NCORES)), trace=trace, tmpdir=tmpdir)
    out = np.empty((B, S, HID), dtype=np.float32)
    for b in range(B):
        lo, hi = res.results[2 * b]["out"], res.results[2 * b + 1]["out"]
        for j in range(NJ):
            out[b, 512 * j:512 * j + 256] = lo[256 * j:256 * (j + 1)]
            out[b, 512 * j + 256:512 * (j + 1)] = hi[256 * j:256 * (j + 1)]
    return out, res


def kernel(**inputs):
    out, _ = run(inputs)
    return out


# revision 15
# speedup vs baseline: 1.0717x; 1.0434x over previous
"""Causal multi-head attention (B=4, S=2048, HID=1024, 16 heads x 64) with RoPE
on 8 TRN2 NeuronCores.

Sharding: core c -> batch b = c//2, head-group hg = c%2 (8 heads each).

Per core: projections in transposed layout (head dims on partitions), RoPE
rotation realized as a strided-partition DMA permutation (sign folded into the
sin table), scores computed transposed sT[kk, q] with both heads of a pair
row-packed into one PSUM tile, exp on ACT with fused scale, causal masking via
affine_select (exact zeros), V augmented with a ones column so the softmax
denominator appears as ctx psum row 64, deferred normalization (sums -> DRAM
bounce -> broadcast DMA -> fast reciprocal -> multiply), o_proj and a pair
ReduceScatter pipelined per 512-query chunk.

All matmuls run in fp16 (fp32 PSUM accumulation).
"""
import os as _os
import numpy as np
from contextlib import ExitStack

import concourse.bass as bass
import concourse.tile as tile
import concourse.mybir as mybir
from concourse import bacc
from concourse.alu_op_type import AluOpType
from concourse.bass_utils import run_bass_kernel_spmd
import concourse.bass_utils as _bu

LDW_OPT = _os.environ.get("KLDW", "0") == "1"
if LDW_OPT and not getattr(_bu, "_ldw_patched", False):
    _orig_run_command = _bu.run_command
    def _run_command_ldwopt(argv, **kw):
        argv = ["--enable-ldw-opt=true" if a == "--enable-ldw-opt=false" else a
                for a in argv]
        return _orig_run_command(argv, **kw)
    _bu.run_command = _run_command_ldwopt
    _bu._ldw_patched = True

F32 = mybir.dt.float32
F16 = mybir.dt.float16
BF16 = mybir.dt.bfloat16
MM_DT = BF16 if _os.environ.get("KMM", "f16") == "bf16" else F16
ROT_DMA = _os.environ.get("KROT", "dma") == "dma"
AF = mybir.ActivationFunctionType
Alu = AluOpType

B, S, HID = 4, 2048, 1024
NH, HD = 16, 64
SCALE = 1.0 / np.sqrt(HD)
ROPE_BASE = 10000.0
NCORES = 8
HPC = 8          # heads per core
JC = 512         # head dims per core
NJ = 4           # q chunks of 512
NT = 16          # kk tiles of 128
NSC = 4          # s chunks of 512 for projections
NHC = 8          # hid chunks of 128 (contraction)

_PROGRAM = None


def build():
    nc = bacc.Bacc("TRN2", target_bir_lowering=False, debug=False)

    hsT_d = nc.declare_dram_parameter("hsT", [HID, S], MM_DT, isOutput=False)
    wq_d = nc.declare_dram_parameter("wqT", [HID, JC], MM_DT, isOutput=False)
    wk_d = nc.declare_dram_parameter("wkT", [HID, JC], MM_DT, isOutput=False)
    wv_d = nc.declare_dram_parameter("wvT", [HID, JC], MM_DT, isOutput=False)
    if not ROT_DMA:
        wqr_d = nc.declare_dram_parameter("wqrT", [HID, JC], MM_DT, isOutput=False)
        wkr_d = nc.declare_dram_parameter("wkrT", [HID, JC], MM_DT, isOutput=False)
    wo_d = nc.declare_dram_parameter("woT", [JC, HID], MM_DT, isOutput=False)
    cos_d = nc.declare_dram_parameter("cosT2", [128, S], MM_DT, isOutput=False)
    sin_d = nc.declare_dram_parameter("sinT2", [128, S], MM_DT, isOutput=False)
    out_d = nc.declare_dram_parameter("out", [S // 2, HID], F32, isOutput=True)

    rdram = nc.dram_tensor("rdram_l" if LDW_OPT else "rdram", [HPC, S], F32)
    cc_in = nc.dram_tensor("cc_in", [S, HID], F32)
    cc_out = [nc.dram_tensor(f"cc_out{j}", [S // 8, HID], F32) for j in range(NJ)]

    with ExitStack() as ctx:
        tc = ctx.enter_context(tile.TileContext(nc, num_cores=NCORES))
        consts = ctx.enter_context(tc.tile_pool(name="consts", bufs=1))
        rt = ctx.enter_context(tc.tile_pool(name="rt", bufs=6))
        ptp = ctx.enter_context(tc.tile_pool(name="ptp", bufs=6))
        misc = ctx.enter_context(tc.tile_pool(name="misc", bufs=2))
        outp = ctx.enter_context(tc.tile_pool(name="outp", bufs=3))
        psum = ctx.enter_context(tc.tile_pool(name="psum", bufs=2, space="PSUM"))

        # ---- load constants ----
        hsT = consts.tile([128, NHC, S], MM_DT, tag="hsT")
        for hc in range(NHC):
            nc.sync.dma_start(out=hsT[:, hc, :], in_=hsT_d[hc * 128:(hc + 1) * 128, :])
        wsb = {}
        wnames = [("wq", wq_d), ("wk", wk_d), ("wv", wv_d)]
        if not ROT_DMA:
            wnames += [("wqr", wqr_d), ("wkr", wkr_d)]
        for name, d in wnames:
            t = consts.tile([128, NHC, JC], MM_DT, tag=name, name=f"w_{name}")
            for hc in range(NHC):
                nc.sync.dma_start(out=t[:, hc, :], in_=d[hc * 128:(hc + 1) * 128, :])
            wsb[name] = t
        wo = consts.tile([128, 4, HID], MM_DT, tag="wo")
        nc.sync.dma_start(out=wo[:], in_=wo_d[:].rearrange("(c p) j -> p c j", p=128))
        cos2 = consts.tile([128, S], MM_DT, tag="cos2")
        sin2 = consts.tile([128, S], MM_DT, tag="sin2")
        nc.sync.dma_start(out=cos2[:], in_=cos_d[:])
        nc.sync.dma_start(out=sin2[:], in_=sin_d[:])

        # ---- projections + RoPE for ALL head pairs up front ----
        qrope = [consts.tile([128, S], MM_DT, tag=f"qrope{i}", name=f"qrope{i}")
                 for i in range(4)]
        krope = [consts.tile([128, S], MM_DT, tag=f"krope{i}", name=f"krope{i}")
                 for i in range(4)]

        for hp in range(4):
            jcol = hp * 128
            for wname, raww, dest in (("wq", "qraw", qrope), ("wk", "kraw", krope)):
                for sc in range(NSC):
                    ssl = slice(sc * 512, (sc + 1) * 512)
                    ps_raw = psum.tile([128, 512], F32, tag="mm")
                    for hc in range(NHC):
                        nc.tensor.matmul(
                            out=ps_raw[:],
                            lhsT=wsb[wname][:, hc, jcol:jcol + 128],
                            rhs=hsT[:, hc, ssl],
                            start=(hc == 0), stop=(hc == NHC - 1),
                        )
                    if ROT_DMA:
                        raw_sb = misc.tile([128, 512], MM_DT, tag="qraw", bufs=4,
                                           name=f"raw_{raww}{hp}_{sc}")
                        nc.scalar.copy(out=raw_sb[:], in_=ps_raw[:])
                        rot_sb = misc.tile([128, 512], MM_DT, tag="qrot", bufs=4,
                                           name=f"rot_{raww}{hp}_{sc}")
                        for hl in range(2):
                            b0 = 64 * hl
                            # rot rows 0:32 <- raw rows 1,3,..,63 (odd)
                            nc.sync.dma_start(
                                out=rot_sb[b0:b0 + 32, :],
                                in_=raw_sb[b0 + 1:b0 + 64:2, :],
                            )
                            # rot rows 32:64 <- raw rows 0,2,..,62 (even)
                            nc.sync.dma_start(
                                out=rot_sb[b0 + 32:b0 + 64, :],
                                in_=raw_sb[b0:b0 + 63:2, :],
                            )
                        t1 = rt.tile([128, 512], MM_DT, tag="rt")
                        t2 = rt.tile([128, 512], MM_DT, tag="rt")
                        nc.vector.tensor_tensor(out=t1[:], in0=raw_sb[:], in1=cos2[:, ssl], op=Alu.mult)
                        nc.vector.tensor_tensor(out=t2[:], in0=rot_sb[:], in1=sin2[:, ssl], op=Alu.mult)
                        nc.vector.tensor_add(out=dest[hp][:, ssl], in0=t1[:], in1=t2[:])
                    else:
                        rot_w = {"wq": "wqr", "wk": "wkr"}[wname]
                        ps_rot = psum.tile([128, 512], F32, tag="mm")
                        for hc in range(NHC):
                            nc.tensor.matmul(
                                out=ps_rot[:],
                                lhsT=wsb[rot_w][:, hc, jcol:jcol + 128],
                                rhs=hsT[:, hc, ssl],
                                start=(hc == 0), stop=(hc == NHC - 1),
                            )
                        t1 = rt.tile([128, 512], F32, tag="rt")
                        t2 = rt.tile([128, 512], F32, tag="rt")
                        nc.vector.tensor_tensor(out=t1[:], in0=ps_raw[:], in1=cos2[:, ssl], op=Alu.mult)
                        nc.vector.tensor_tensor(out=t2[:], in0=ps_rot[:], in1=sin2[:, ssl], op=Alu.mult)
                        nc.vector.tensor_add(out=dest[hp][:, ssl], in0=t1[:], in1=t2[:])

        # ---- V for all heads, natural layout + ones column ----
        v_sb = consts.tile([128, NT, HPC, HD + 1], MM_DT, tag="v_sb")
        nc.vector.memset(v_sb[:, :, :, HD:HD + 1], 1.0)
        for st in range(NT):
            v_ps = psum.tile([128, JC], F32, tag="mm")
            for hc in range(NHC):
                nc.tensor.matmul(
                    out=v_ps[:],
                    lhsT=hsT[:, hc, st * 128:(st + 1) * 128],
                    rhs=wsb["wv"][:, hc, :],
                    start=(hc == 0), stop=(hc == NHC - 1),
                )
            nc.vector.tensor_copy(
                out=v_sb[:, st, :, 0:HD],
                in_=v_ps[:].rearrange("p (h d) -> p h d", h=HPC),
            )

        # ---- attention + o_proj + collective, pipelined per q-chunk ----
        ctx_sb = [consts.tile([128, S], MM_DT, tag=f"ctx{i}", name=f"ctx_sb{i}")
                  for i in range(4)]

        def attn_block(j):
            qsl = slice(j * 512, (j + 1) * 512)
            for hp in range(4):
                ctx_ps = [psum.tile([HD + 1, 512], F32, tag="ctx", name=f"ctx_ps{_i}")
                          for _i in range(2)]
                nt = 4 * j + 4
                for t in range(nt):
                    sc_ps = psum.tile([128, 2, 512], F32, tag="sc")
                    ksl = slice(t * 128, (t + 1) * 128)
                    for hl in range(2):
                        pr = slice(64 * hl, 64 * hl + 64)
                        nc.tensor.matmul(
                            out=sc_ps[:, hl, :],
                            lhsT=krope[hp][pr, ksl],
                            rhs=qrope[hp][pr, qsl],
                            start=True, stop=True,
                        )
                    pt = ptp.tile([128, 2, 512], MM_DT, tag="pt")
                    nc.scalar.activation(out=pt[:], in_=sc_ps[:], func=AF.Exp, scale=float(SCALE))
                    if t >= 4 * j:
                        for hl in range(2):
                            nc.gpsimd.affine_select(
                                out=pt[:, hl, :], in_=pt[:, hl, :],
                                pattern=[[1, 512]], compare_op=Alu.is_ge,
                                fill=0.0, base=512 * j - 128 * t,
                                channel_multiplier=-1,
                            )
                    for hl in range(2):
                        nc.tensor.matmul(
                            out=ctx_ps[hl][:],
                            lhsT=v_sb[:, t, 2 * hp + hl, :],
                            rhs=pt[:, hl, :],
                            start=(t == 0), stop=(t == nt - 1),
                        )
                for hl in range(2):
                    pr = slice(64 * hl, 64 * hl + 64)
                    nc.vector.tensor_copy(out=ctx_sb[hp][pr, qsl], in_=ctx_ps[hl][0:HD, :])
                    srow = misc.tile([128, 512], F32, tag="srow")
                    nc.vector.tensor_copy(out=srow[64:65, :], in_=ctx_ps[hl][HD:HD + 1, :])
                    nc.sync.dma_start(
                        out=rdram[2 * hp + hl:2 * hp + hl + 1, qsl],
                        in_=srow[64:65, :],
                    )
        def fin_block(j):
            qsl = slice(j * 512, (j + 1) * 512)
            # normalize all head pairs for this q chunk
            for hp in range(4):
                bc = misc.tile([128, 512], F32, tag="bc")
                nc.sync.dma_start(
                    out=bc[0:64, :],
                    in_=rdram[2 * hp:2 * hp + 1, qsl].partition_broadcast(64),
                )
                nc.sync.dma_start(
                    out=bc[64:128, :],
                    in_=rdram[2 * hp + 1:2 * hp + 2, qsl].partition_broadcast(64),
                )
                nc.vector.reciprocal_approx_fast(out=bc[:], in_=bc[:])
                nc.vector.tensor_tensor(
                    out=ctx_sb[hp][:, qsl], in0=ctx_sb[hp][:, qsl], in1=bc[:], op=Alu.mult,
                )
            # o_proj rows of this q chunk
            for st in range(4 * j, 4 * j + 4):
                ssl2 = slice(st * 128, (st + 1) * 128)
                for jc2 in range(2):
                    osl = slice(jc2 * 512, (jc2 + 1) * 512)
                    o_ps = psum.tile([128, 512], F32, tag="mm")
                    for kc in range(4):
                        nc.tensor.matmul(
                            out=o_ps[:],
                            lhsT=ctx_sb[kc][:, ssl2],
                            rhs=wo[:, kc, osl],
                            start=(kc == 0), stop=(kc == 3),
                        )
                    o_sb = outp.tile([128, 512], F32, tag="osb")
                    nc.vector.tensor_copy(out=o_sb[:], in_=o_ps[:])
                    nc.sync.dma_start(out=cc_in[ssl2, osl], in_=o_sb[:])
            # pair ReduceScatter for this q chunk (last chunk split in two
            # to shrink the un-overlapped tail)
            if j < NJ - 1:
                nc.gpsimd.collective_compute(
                    "ReduceScatter", Alu.add,
                    replica_groups=[[0, 1], [2, 3], [4, 5], [6, 7]],
                    ins=[cc_in[j * 512:(j + 1) * 512, :]], outs=[cc_out[j][:]],
                )
            else:
                nc.gpsimd.collective_compute(
                    "ReduceScatter", Alu.add,
                    replica_groups=[[0, 1], [2, 3], [4, 5], [6, 7]],
                    ins=[cc_in[j * 512:j * 512 + 256, :]], outs=[cc_out[j][0:128, :]],
                )
                nc.gpsimd.collective_compute(
                    "ReduceScatter", Alu.add,
                    replica_groups=[[0, 1], [2, 3], [4, 5], [6, 7]],
                    ins=[cc_in[j * 512 + 256:(j + 1) * 512, :]], outs=[cc_out[j][128:256, :]],
                )

        attn_block(0)
        attn_block(1)
        fin_block(0)
        attn_block(2)
        fin_block(1)
        attn_block(3)
        fin_block(2)
        fin_block(3)

        for j in range(NJ):
            nc.sync.dma_start(
                out=out_d[j * 256:(j + 1) * 256, :], in_=cc_out[j][:],
            )

    nc.finalize()
    return nc


def _rope_tables():
    inv_freq = (1.0 / (ROPE_BASE ** (np.arange(0, HD, 2, dtype=np.float32) / np.float32(HD)))).astype(np.float32)
    t = np.arange(S, dtype=np.float32)
    freqs = np.outer(t, inv_freq).astype(np.float32)          # [S, 32]
    emb = np.concatenate([freqs, freqs], axis=-1)             # [S, 64]
    return np.cos(emb).astype(np.float32), np.sin(emb).astype(np.float32)


def _rot_weights(W):
    """Rows of Wr give rotated(x) = cat(-x2, x1) of x = W @ h per 64-dim head."""
    Wr = np.empty_like(W)
    for h in range(NH):
        b = h * HD
        Wr[b:b + 32] = -W[b + 1:b + HD:2]
        Wr[b + 32:b + HD] = W[b:b + HD:2]
    return Wr


def prepare_in_maps(hidden_states, Wq, Wk, Wv, Wo):
    cos, sin = _rope_tables()                                  # [S, 64]
    cos2 = np.ascontiguousarray(np.tile(cos.T, (2, 1)))        # [128, S]
    sin2 = np.ascontiguousarray(np.tile(sin.T, (2, 1)))
    if ROT_DMA:
        # sign of the rotation (-x2 for d<32) folded into the sin table
        sin2[0:32] *= -1.0
        sin2[64:96] *= -1.0
    else:
        Wqr = _rot_weights(Wq)
        Wkr = _rot_weights(Wk)
    if MM_DT == F16:
        f16 = np.float16
    else:
        import ml_dtypes
        f16 = ml_dtypes.bfloat16
    in_maps = []
    for c in range(NCORES):
        b, hg = c // 2, c % 2
        sl = slice(JC * hg, JC * (hg + 1))
        m = {
            "hsT": np.ascontiguousarray(hidden_states[b].T).astype(f16),
            "wqT": np.ascontiguousarray(Wq[sl].T).astype(f16),
            "wkT": np.ascontiguousarray(Wk[sl].T).astype(f16),
            "wvT": np.ascontiguousarray(Wv[sl].T).astype(f16),
            "woT": np.ascontiguousarray(Wo[:, sl].T).astype(f16),
            "cosT2": cos2.astype(f16),
            "sinT2": sin2.astype(f16),
        }
        if not ROT_DMA:
            m["wqrT"] = np.ascontiguousarray(Wqr[sl].T).astype(f16)
            m["wkrT"] = np.ascontiguousarray(Wkr[sl].T).astype(f16)
        in_maps.append(m)
    return in_maps


def run(inputs, trace=False, tmpdir=None):
    global _PROGRAM
    if _PROGRAM is None:
        _PROGRAM = build()
    nc = _PROGRAM
    in_maps = prepare_in_maps(
        np.asarray(inputs["hidden_states"], dtype=np.float32),
        np.asarray(inputs["Wq"], dtype=np.float32),
        np.asarray(inputs["Wk"], dtype=np.float32),
        np.asarray(inputs["Wv"], dtype=np.float32),
        np.asarray(inputs["Wo"], dtype=np.float32),
    )
    res = run_bass_kernel_spmd(nc, in_maps, list(range(NCORES)), trace=trace, tmpdir=tmpdir)
    out = np.empty((B, S, HID), dtype=np.float32)
    for b in range(B):
        lo, hi = res.results[2 * b]["out"], res.results[2 * b + 1]["out"]
        for j in range(NJ - 1):
            out[b, 512 * j:512 * j + 256] = lo[256 * j:256 * (j + 1)]
            out[b, 512 * j + 256:512 * (j + 1)] = hi[256 * j:256 * (j + 1)]
        # last chunk was reduce-scattered in two 256-row halves
        out[b, 1536:1664] = lo[768:896]
        out[b, 1664:1792] = hi[768:896]
        out[b, 1792:1920] = lo[896:1024]
        out[b, 1920:2048] = hi[896:1024]
    return out, res


def kernel(**inputs):
    out, _ = run(inputs)
    return out


# revision 16
# speedup vs baseline: 1.0807x; 1.0084x over previous
"""Causal multi-head attention (B=4, S=2048, HID=1024, 16 heads x 64) with RoPE
on 8 TRN2 NeuronCores.

Sharding: core c -> batch b = c//2, head-group hg = c%2 (8 heads each).

Per core: projections in transposed layout (head dims on partitions), RoPE
rotation realized as a strided-partition DMA permutation (sign folded into the
sin table), scores computed transposed sT[kk, q] with both heads of a pair
row-packed into one PSUM tile, exp on ACT with fused scale, causal masking via
affine_select (exact zeros), V augmented with a ones column so the softmax
denominator appears as ctx psum row 64, deferred normalization (sums -> DRAM
bounce -> broadcast DMA -> fast reciprocal -> multiply), o_proj and a pair
ReduceScatter pipelined per 512-query chunk.

All matmuls run in fp16 (fp32 PSUM accumulation).
"""
import os as _os
import numpy as np
from contextlib import ExitStack

import concourse.bass as bass
import concourse.tile as tile
import concourse.mybir as mybir
from concourse import bacc
from concourse.alu_op_type import AluOpType
from concourse.bass_utils import run_bass_kernel_spmd
import concourse.bass_utils as _bu

LDW_OPT = _os.environ.get("KLDW", "0") == "1"
if LDW_OPT and not getattr(_bu, "_ldw_patched", False):
    _orig_run_command = _bu.run_command
    def _run_command_ldwopt(argv, **kw):
        argv = ["--enable-ldw-opt=true" if a == "--enable-ldw-opt=false" else a
                for a in argv]
        return _orig_run_command(argv, **kw)
    _bu.run_command = _run_command_ldwopt
    _bu._ldw_patched = True

F32 = mybir.dt.float32
F16 = mybir.dt.float16
BF16 = mybir.dt.bfloat16
MM_DT = BF16 if _os.environ.get("KMM", "f16") == "bf16" else F16
ROT_DMA = _os.environ.get("KROT", "dma") == "dma"
AF = mybir.ActivationFunctionType
Alu = AluOpType

B, S, HID = 4, 2048, 1024
NH, HD = 16, 64
SCALE = 1.0 / np.sqrt(HD)
ROPE_BASE = 10000.0
NCORES = 8
HPC = 8          # heads per core
JC = 512         # head dims per core
NJ = 4           # q chunks of 512
NT = 16          # kk tiles of 128
NSC = 4          # s chunks of 512 for projections
NHC = 8          # hid chunks of 128 (contraction)

_PROGRAM = None


def build():
    nc = bacc.Bacc("TRN2", target_bir_lowering=False, debug=False)

    hsT_d = nc.declare_dram_parameter("hsT", [HID, S], MM_DT, isOutput=False)
    wq_d = nc.declare_dram_parameter("wqT", [HID, JC], MM_DT, isOutput=False)
    wk_d = nc.declare_dram_parameter("wkT", [HID, JC], MM_DT, isOutput=False)
    wv_d = nc.declare_dram_parameter("wvT", [HID, JC], MM_DT, isOutput=False)
    if not ROT_DMA:
        wqr_d = nc.declare_dram_parameter("wqrT", [HID, JC], MM_DT, isOutput=False)
        wkr_d = nc.declare_dram_parameter("wkrT", [HID, JC], MM_DT, isOutput=False)
    wo_d = nc.declare_dram_parameter("woT", [JC, HID], MM_DT, isOutput=False)
    cos_d = nc.declare_dram_parameter("cosT2", [128, S], MM_DT, isOutput=False)
    sin_d = nc.declare_dram_parameter("sinT2", [128, S], MM_DT, isOutput=False)
    out_d = nc.declare_dram_parameter("out", [S // 2, HID], F32, isOutput=True)

    rdram = nc.dram_tensor("rdram_l" if LDW_OPT else "rdram", [HPC, S], F32)
    cc_in = nc.dram_tensor("cc_in", [S, HID], F32)
    cc_out = [nc.dram_tensor(f"cc_out{j}", [S // 8, HID], F32) for j in range(NJ)]

    with ExitStack() as ctx:
        tc = ctx.enter_context(tile.TileContext(nc, num_cores=NCORES))
        consts = ctx.enter_context(tc.tile_pool(name="consts", bufs=1))
        rt = ctx.enter_context(tc.tile_pool(name="rt", bufs=6))
        ptp = ctx.enter_context(tc.tile_pool(name="ptp", bufs=6))
        misc = ctx.enter_context(tc.tile_pool(name="misc", bufs=2))
        outp = ctx.enter_context(tc.tile_pool(name="outp", bufs=3))
        psum = ctx.enter_context(tc.tile_pool(name="psum", bufs=2, space="PSUM"))

        # ---- load constants ----
        hsT = consts.tile([128, NHC, S], MM_DT, tag="hsT")
        for hc in range(NHC):
            nc.sync.dma_start(out=hsT[:, hc, :], in_=hsT_d[hc * 128:(hc + 1) * 128, :])
        wsb = {}
        wnames = [("wq", wq_d), ("wk", wk_d), ("wv", wv_d)]
        if not ROT_DMA:
            wnames += [("wqr", wqr_d), ("wkr", wkr_d)]
        for name, d in wnames:
            t = consts.tile([128, NHC, JC], MM_DT, tag=name, name=f"w_{name}")
            for hc in range(NHC):
                nc.sync.dma_start(out=t[:, hc, :], in_=d[hc * 128:(hc + 1) * 128, :])
            wsb[name] = t
        wo = consts.tile([128, 4, HID], MM_DT, tag="wo")
        nc.sync.dma_start(out=wo[:], in_=wo_d[:].rearrange("(c p) j -> p c j", p=128))
        cos2 = consts.tile([128, S], MM_DT, tag="cos2")
        sin2 = consts.tile([128, S], MM_DT, tag="sin2")
        nc.sync.dma_start(out=cos2[:], in_=cos_d[:])
        nc.sync.dma_start(out=sin2[:], in_=sin_d[:])

        # ---- projections + RoPE for ALL head pairs up front ----
        qrope = [consts.tile([128, S], MM_DT, tag=f"qrope{i}", name=f"qrope{i}")
                 for i in range(4)]
        krope = [consts.tile([128, S], MM_DT, tag=f"krope{i}", name=f"krope{i}")
                 for i in range(4)]

        for hp in range(4):
            jcol = hp * 128
            for wname, raww, dest in (("wq", "qraw", qrope), ("wk", "kraw", krope)):
                for sc in range(NSC):
                    ssl = slice(sc * 512, (sc + 1) * 512)
                    ps_raw = psum.tile([128, 512], F32, tag="mm")
                    for hc in range(NHC):
                        nc.tensor.matmul(
                            out=ps_raw[:],
                            lhsT=wsb[wname][:, hc, jcol:jcol + 128],
                            rhs=hsT[:, hc, ssl],
                            start=(hc == 0), stop=(hc == NHC - 1),
                        )
                    if ROT_DMA:
                        raw_sb = misc.tile([128, 512], MM_DT, tag="qraw", bufs=4,
                                           name=f"raw_{raww}{hp}_{sc}")
                        nc.scalar.copy(out=raw_sb[:], in_=ps_raw[:])
                        rot_sb = misc.tile([128, 512], MM_DT, tag="qrot", bufs=4,
                                           name=f"rot_{raww}{hp}_{sc}")
                        for hl in range(2):
                            b0 = 64 * hl
                            # rot rows 0:32 <- raw rows 1,3,..,63 (odd)
                            nc.sync.dma_start(
                                out=rot_sb[b0:b0 + 32, :],
                                in_=raw_sb[b0 + 1:b0 + 64:2, :],
                            )
                            # rot rows 32:64 <- raw rows 0,2,..,62 (even)
                            nc.sync.dma_start(
                                out=rot_sb[b0 + 32:b0 + 64, :],
                                in_=raw_sb[b0:b0 + 63:2, :],
                            )
                        t1 = rt.tile([128, 512], MM_DT, tag="rt")
                        t2 = rt.tile([128, 512], MM_DT, tag="rt")
                        nc.vector.tensor_tensor(out=t1[:], in0=raw_sb[:], in1=cos2[:, ssl], op=Alu.mult)
                        nc.vector.tensor_tensor(out=t2[:], in0=rot_sb[:], in1=sin2[:, ssl], op=Alu.mult)
                        nc.vector.tensor_add(out=dest[hp][:, ssl], in0=t1[:], in1=t2[:])
                    else:
                        rot_w = {"wq": "wqr", "wk": "wkr"}[wname]
                        ps_rot = psum.tile([128, 512], F32, tag="mm")
                        for hc in range(NHC):
                            nc.tensor.matmul(
                                out=ps_rot[:],
                                lhsT=wsb[rot_w][:, hc, jcol:jcol + 128],
                                rhs=hsT[:, hc, ssl],
                                start=(hc == 0), stop=(hc == NHC - 1),
                            )
                        t1 = rt.tile([128, 512], F32, tag="rt")
                        t2 = rt.tile([128, 512], F32, tag="rt")
                        nc.vector.tensor_tensor(out=t1[:], in0=ps_raw[:], in1=cos2[:, ssl], op=Alu.mult)
                        nc.vector.tensor_tensor(out=t2[:], in0=ps_rot[:], in1=sin2[:, ssl], op=Alu.mult)
                        nc.vector.tensor_add(out=dest[hp][:, ssl], in0=t1[:], in1=t2[:])

        # ---- V for all heads, natural layout + ones column ----
        v_sb = consts.tile([128, NT, HPC, HD + 1], MM_DT, tag="v_sb")
        nc.vector.memset(v_sb[:, :, :, HD:HD + 1], 1.0)
        for st in range(NT):
            v_ps = psum.tile([128, JC], F32, tag="mm")
            for hc in range(NHC):
                nc.tensor.matmul(
                    out=v_ps[:],
                    lhsT=hsT[:, hc, st * 128:(st + 1) * 128],
                    rhs=wsb["wv"][:, hc, :],
                    start=(hc == 0), stop=(hc == NHC - 1),
                )
            nc.vector.tensor_copy(
                out=v_sb[:, st, :, 0:HD],
                in_=v_ps[:].rearrange("p (h d) -> p h d", h=HPC),
            )

        # ---- attention + o_proj + collective, pipelined per q-chunk ----
        ctx_sb = [consts.tile([128, S], MM_DT, tag=f"ctx{i}", name=f"ctx_sb{i}")
                  for i in range(4)]

        def attn_block(j):
            qsl = slice(j * 512, (j + 1) * 512)
            for hp in range(4):
                ctx_ps = [psum.tile([HD + 1, 512], F32, tag="ctx", name=f"ctx_ps{_i}")
                          for _i in range(2)]
                nt = 4 * j + 4
                for t in range(nt):
                    sc_ps = psum.tile([128, 2, 512], F32, tag="sc")
                    ksl = slice(t * 128, (t + 1) * 128)
                    for hl in range(2):
                        pr = slice(64 * hl, 64 * hl + 64)
                        nc.tensor.matmul(
                            out=sc_ps[:, hl, :],
                            lhsT=krope[hp][pr, ksl],
                            rhs=qrope[hp][pr, qsl],
                            start=True, stop=True,
                        )
                    pt = ptp.tile([128, 2, 512], MM_DT, tag="pt")
                    nc.scalar.activation(out=pt[:], in_=sc_ps[:], func=AF.Exp, scale=float(SCALE))
                    if t >= 4 * j:
                        for hl in range(2):
                            nc.gpsimd.affine_select(
                                out=pt[:, hl, :], in_=pt[:, hl, :],
                                pattern=[[1, 512]], compare_op=Alu.is_ge,
                                fill=0.0, base=512 * j - 128 * t,
                                channel_multiplier=-1,
                            )
                    for hl in range(2):
                        nc.tensor.matmul(
                            out=ctx_ps[hl][:],
                            lhsT=v_sb[:, t, 2 * hp + hl, :],
                            rhs=pt[:, hl, :],
                            start=(t == 0), stop=(t == nt - 1),
                        )
                for hl in range(2):
                    pr = slice(64 * hl, 64 * hl + 64)
                    nc.vector.tensor_copy(out=ctx_sb[hp][pr, qsl], in_=ctx_ps[hl][0:HD, :])
                    srow = misc.tile([128, 512], F32, tag="srow")
                    nc.vector.tensor_copy(out=srow[64:65, :], in_=ctx_ps[hl][HD:HD + 1, :])
                    nc.sync.dma_start(
                        out=rdram[2 * hp + hl:2 * hp + hl + 1, qsl],
                        in_=srow[64:65, :],
                    )
        def fin_block(j):
            qsl = slice(j * 512, (j + 1) * 512)
            # normalize all head pairs for this q chunk
            for hp in range(4):
                bc = misc.tile([128, 512], F32, tag="bc")
                nc.sync.dma_start(
                    out=bc[0:64, :],
                    in_=rdram[2 * hp:2 * hp + 1, qsl].partition_broadcast(64),
                )
                nc.sync.dma_start(
                    out=bc[64:128, :],
                    in_=rdram[2 * hp + 1:2 * hp + 2, qsl].partition_broadcast(64),
                )
                nc.vector.reciprocal_approx_fast(out=bc[:], in_=bc[:])
                nc.vector.tensor_tensor(
                    out=ctx_sb[hp][:, qsl], in0=ctx_sb[hp][:, qsl], in1=bc[:], op=Alu.mult,
                )
            # o_proj rows of this q chunk
            for st in range(4 * j, 4 * j + 4):
                ssl2 = slice(st * 128, (st + 1) * 128)
                for jc2 in range(2):
                    osl = slice(jc2 * 512, (jc2 + 1) * 512)
                    o_ps = psum.tile([128, 512], F32, tag="mm")
                    for kc in range(4):
                        nc.tensor.matmul(
                            out=o_ps[:],
                            lhsT=ctx_sb[kc][:, ssl2],
                            rhs=wo[:, kc, osl],
                            start=(kc == 0), stop=(kc == 3),
                        )
                    o_sb = outp.tile([128, 512], F32, tag="osb")
                    nc.vector.tensor_copy(out=o_sb[:], in_=o_ps[:])
                    nc.sync.dma_start(out=cc_in[ssl2, osl], in_=o_sb[:])
        def rs_block(j):
            # pair ReduceScatter for this q chunk (last chunk split in two
            # to shrink the un-overlapped tail)
            if j < NJ - 1:
                nc.gpsimd.collective_compute(
                    "ReduceScatter", Alu.add,
                    replica_groups=[[0, 1], [2, 3], [4, 5], [6, 7]],
                    ins=[cc_in[j * 512:(j + 1) * 512, :]], outs=[cc_out[j][:]],
                )
            else:
                nc.gpsimd.collective_compute(
                    "ReduceScatter", Alu.add,
                    replica_groups=[[0, 1], [2, 3], [4, 5], [6, 7]],
                    ins=[cc_in[j * 512:j * 512 + 256, :]], outs=[cc_out[j][0:128, :]],
                )
                nc.gpsimd.collective_compute(
                    "ReduceScatter", Alu.add,
                    replica_groups=[[0, 1], [2, 3], [4, 5], [6, 7]],
                    ins=[cc_in[j * 512 + 256:(j + 1) * 512, :]], outs=[cc_out[j][128:256, :]],
                )

        # Software-pipelined emission: attention(j+1) precedes finalize(j) so
        # the PE queue never head-of-line blocks on the normalize round-trip,
        # and each RS trigger sits after the NEXT attention block's gpsimd
        # work so its data-ready wait never stalls affine_selects.
        attn_block(0)
        attn_block(1)
        fin_block(0)
        attn_block(2)
        rs_block(0)
        fin_block(1)
        attn_block(3)
        rs_block(1)
        fin_block(2)
        rs_block(2)
        fin_block(3)
        rs_block(3)

        for j in range(NJ):
            nc.sync.dma_start(
                out=out_d[j * 256:(j + 1) * 256, :], in_=cc_out[j][:],
            )

    nc.finalize()
    return nc


def _rope_tables():
    inv_freq = (1.0 / (ROPE_BASE ** (np.arange(0, HD, 2, dtype=np.float32) / np.float32(HD)))).astype(np.float32)
    t = np.arange(S, dtype=np.float32)
    freqs = np.outer(t, inv_freq).astype(np.float32)          # [S, 32]
    emb = np.concatenate([freqs, freqs], axis=-1)             # [S, 64]
    return np.cos(emb).astype(np.float32), np.sin(emb).astype(np.float32)


def _rot_weights(W):
    """Rows of Wr give rotated(x) = cat(-x2, x1) of x = W @ h per 64-dim head."""
    Wr = np.empty_like(W)
    for h in range(NH):
        b = h * HD
        Wr[b:b + 32] = -W[b + 1:b + HD:2]
        Wr[b + 32:b + HD] = W[b:b + HD:2]
    return Wr


def prepare_in_maps(hidden_states, Wq, Wk, Wv, Wo):
    cos, sin = _rope_tables()                                  # [S, 64]
    cos2 = np.ascontiguousarray(np.tile(cos.T, (2, 1)))        # [128, S]
    sin2 = np.ascontiguousarray(np.tile(sin.T, (2, 1)))
    if ROT_DMA:
        # sign of the rotation (-x2 for d<32) folded into the sin table
        sin2[0:32] *= -1.0
        sin2[64:96] *= -1.0
    else:
        Wqr = _rot_weights(Wq)
        Wkr = _rot_weights(Wk)
    if MM_DT == F16:
        f16 = np.float16
    else:
        import ml_dtypes
        f16 = ml_dtypes.bfloat16
    in_maps = []
    for c in range(NCORES):
        b, hg = c // 2, c % 2
        sl = slice(JC * hg, JC * (hg + 1))
        m = {
            "hsT": np.ascontiguousarray(hidden_states[b].T).astype(f16),
            "wqT": np.ascontiguousarray(Wq[sl].T).astype(f16),
            "wkT": np.ascontiguousarray(Wk[sl].T).astype(f16),
            "wvT": np.ascontiguousarray(Wv[sl].T).astype(f16),
            "woT": np.ascontiguousarray(Wo[:, sl].T).astype(f16),
            "cosT2": cos2.astype(f16),
            "sinT2": sin2.astype(f16),
        }
        if not ROT_DMA:
            m["wqrT"] = np.ascontiguousarray(Wqr[sl].T).astype(f16)
            m["wkrT"] = np.ascontiguousarray(Wkr[sl].T).astype(f16)
        in_maps.append(m)
    return in_maps


def run(inputs, trace=False, tmpdir=None):
    global _PROGRAM
    if _PROGRAM is None:
        _PROGRAM = build()
    nc = _PROGRAM
    in_maps = prepare_in_maps(
        np.asarray(inputs["hidden_states"], dtype=np.float32),
        np.asarray(inputs["Wq"], dtype=np.float32),
        np.asarray(inputs["Wk"], dtype=np.float32),
        np.asarray(inputs["Wv"], dtype=np.float32),
        np.asarray(inputs["Wo"], dtype=np.float32),
    )
    res = run_bass_kernel_spmd(nc, in_maps, list(range(NCORES)), trace=trace, tmpdir=tmpdir)
    out = np.empty((B, S, HID), dtype=np.float32)
    for b in range(B):
        lo, hi = res.results[2 * b]["out"], res.results[2 * b + 1]["out"]
        for j in range(NJ - 1):
            out[b, 512 * j:512 * j + 256] = lo[256 * j:256 * (j + 1)]
            out[b, 512 * j + 256:512 * (j + 1)] = hi[256 * j:256 * (j + 1)]
        # last chunk was reduce-scattered in two 256-row halves
        out[b, 1536:1664] = lo[768:896]
        out[b, 1664:1792] = hi[768:896]
        out[b, 1792:1920] = lo[896:1024]
        out[b, 1920:2048] = hi[896:1024]
    return out, res


def kernel(**inputs):
    out, _ = run(inputs)
    return out
